# revision 20
# baseline (speedup 1.0000x reference)
"""Multi-head attention (B=2, S=2048, D=1024, H=16) on 8 TRN2 NeuronCores.

Sharding: data-parallel on batch (2) x tensor-parallel on heads (4 groups of
4 heads).  Core c handles batch c//4, heads 4*(c%4) .. 4*(c%4)+3.

Per-core device kernel computes, for its 4 heads:
  QT = (Wq_s @ x_b + bq)      stored [256 dims, 2048 tok]
  KT = (Wk_s @ x_b + bk)      stored [256 dims, 2048 tok]
  V  = (x_b @ Wv_s.T + bv)    stored [2048 tok, 4, 64|1]   (ones column)
  per head h, per query-half qh:
    S.T  = K_h @ Q_h.T               [2048 k, 1024 q] in 16 PSUM chunks
    E.T  = exp(S.T / 8)              in SBUF (fp32r)
    ctxT | colsum = [V_h | 1].T @ E.T   (PSUM accumulate over k-chunks)
    recip = 1/colsum ; broadcast over 128 partitions via rank-1 matmul
    attn.T chunks = E.T * bcast  -> DMA out  (host transposes back)
    ctxT_h = ctxT * bcast[0:64]
  out_partial = ctx @ Wo_s.T  [2048, 1024]  (host sums partials + bo)

All matmuls run in float32r (~14-bit mantissa, PE 1 cyc/row when the moving
free dim >= 256 vs 4 cyc/row for fp32).  fp32->fp32r rounding happens in
gpsimd cast-DMAs (inputs) or on the producing ACT/DVE op (intermediates).
"""

import sys

import numpy as np

sys.path.insert(0, "/opt/trn_rl_repo")

import concourse.bass as bass
import concourse.mybir as mybir
import concourse.tile as tile
from concourse.bass_utils import run_bass_kernel_spmd

B, S, D = 2, 2048, 1024
NHEADS = 16
DK = 64
NCORES = 8
HPC = 4            # heads per core
DH = HPC * DK      # 256 head dims per core
F32 = mybir.dt.float32
F32R = mybir.dt.float32r
F16 = mybir.dt.float16
AF = mybir.ActivationFunctionType

SCALE = 0.125      # 1/sqrt(DK)

KS_D = D // 128    # 8 K-subtiles over the model dim
TQ = S // 512      # 4 moving chunks of 512 tokens
TC = S // 128      # 16 stationary token chunks of 128
KC = S // 128      # 16 key chunks of 128
QH = 2             # query halves
QHW = S // QH      # 1024


def split_multi_waits(nc, max_waits=1):
    """Walrus in this toolchain encodes at most one sync wait per
    instruction; Tile emits 2+ at join points. Peel extra waits onto
    preceding same-engine NoOps (sequential waits == AND of waits)."""
    ctr = 0
    for f in nc.m.functions:
        for blk in f.blocks:
            out = []
            for inst in blk.instructions:
                si = getattr(inst, "sync_info", None)
                ow = list(si.on_wait) if si is not None and si.on_wait else []
                if len(ow) > max_waits:
                    for w in ow[:-max_waits]:
                        ctr += 1
                        out.append(
                            mybir.InstNoOp(
                                name=f"waitsplit-{ctr}",
                                engine=inst.engine,
                                sync_info=mybir.SyncInfo(
                                    on_wait=[w], on_update=[]
                                ),
                            )
                        )
                    inst.sync_info = mybir.SyncInfo(
                        on_wait=ow[-max_waits:], on_update=si.on_update
                    )
                out.append(inst)
            blk.instructions = out
    return nc


def build_nc(split_waits: bool = True) -> bass.Bass:
    nc = bass.Bass()

    xq = nc.declare_dram_parameter("xq", [D, S], F16, isOutput=False)
    xk = nc.declare_dram_parameter("xk", [D, S], F16, isOutput=False)
    xv = nc.declare_dram_parameter("xv", [D, S], F16, isOutput=False)
    wq = nc.declare_dram_parameter("wq", [D, DH], F16, isOutput=False)
    wk = nc.declare_dram_parameter("wk", [D, DH], F16, isOutput=False)
    wv = nc.declare_dram_parameter("wv", [D, DH], F16, isOutput=False)
    wo = nc.declare_dram_parameter("wo", [DH, D], F16, isOutput=False)
    bq = nc.declare_dram_parameter("bq", [DH], F32, isOutput=False)
    bk = nc.declare_dram_parameter("bk", [DH], F32, isOutput=False)
    bv = nc.declare_dram_parameter("bv", [DH], F32, isOutput=False)
    attn_t = nc.declare_dram_parameter("attn_t", [HPC, S, S], F16, isOutput=True)
    out_p = nc.declare_dram_parameter("out_p", [S, D], F16, isOutput=True)

    with tile.TileContext(nc) as tc, \
         nc.allow_low_precision(reason="fp32r matmul operands (~14-bit)"):
        import contextlib

        ctx = contextlib.ExitStack()
        with ctx:
            wpool = ctx.enter_context(tc.tile_pool(name="wpool", bufs=1))
            qkv = ctx.enter_context(tc.tile_pool(name="qkv", bufs=1))

            # ---- persistent weights / constants ----
            wo_sb = wpool.tile([128, DH // 128, D], F16, tag="wo")
            ones_f32 = wpool.tile([1, 128], F32, tag="ones0")
            nc.vector.memset(ones_f32, 1.0)
            ones_sb = wpool.tile([1, 128], F32R, tag="ones")
            nc.vector.tensor_copy(out=ones_sb, in_=ones_f32)
            one_col = wpool.tile([128, 1], F32, tag="onec")
            nc.vector.memset(one_col, 1.0)

            # ---- persistent activations (split for finer dep granularity) ----
            qt_sb = [
                [
                    qkv.tile([128, QHW], F32R, tag=f"qt{m}{q}", name=f"qt{m}{q}")
                    for q in range(QH)
                ]
                for m in range(2)
            ]
            kt_sb = [qkv.tile([128, S], F32R, tag=f"kt{m}", name=f"kt{m}") for m in range(2)]
            v_sb = [
                qkv.tile([128, HPC, DK + 1], F16, tag=f"v{t}", name=f"v{t}")
                for t in range(TC)
            ]
            ctx_sb = qkv.tile([128, 2, S], F16, tag="ctx")

            # ---- projections ----
            with tc.tile_pool(name="xin", bufs=3) as xin, \
                 tc.tile_pool(name="pw", bufs=1) as pw, \
                 tc.tile_pool(name="pj_ps", bufs=2, space="PSUM") as pj_ps:

                wq_sb = pw.tile([128, KS_D, DH], F16, tag="wq")
                wk_sb = pw.tile([128, KS_D, DH], F16, tag="wk")
                wv_sb = pw.tile([128, KS_D, DH], F16, tag="wv")
                for w_dram, w_sb in ((wk, wk_sb), (wq, wq_sb), (wv, wv_sb)):
                    nc.sync.dma_start(
                        out=w_sb, in_=w_dram.rearrange("(k p) m -> p k m", p=128)
                    )
                bq_sb = pw.tile([128, 2], F32, tag="bq")
                bk_sb = pw.tile([128, 2], F32, tag="bk")
                nc.sync.dma_start(out=bq_sb, in_=bq.rearrange("(m p) -> p m", p=128))
                nc.sync.dma_start(out=bk_sb, in_=bk.rearrange("(m p) -> p m", p=128))
                bvb = pw.tile([128, DH], F32, tag="bvb")
                nc.sync.dma_start(out=bvb, in_=bv[:].partition_broadcast(128))
                nc.sync.dma_start(
                    out=wo_sb, in_=wo.rearrange("(k p) n -> p k n", p=128)
                )

                def qk_out(qk_sb, m, t):
                    # kt: per-m [128, S] tiles; qt: per-(m, qh) [128, QHW]
                    if isinstance(qk_sb[m], list):
                        half = (t * 512) // QHW
                        off = (t * 512) % QHW
                        return qk_sb[m][half][:, off : off + 512]
                    return qk_sb[m][:, t * 512 : (t + 1) * 512]

                for xdram, w_sb, b_sb, qk_sb in (
                    (xk, wk_sb, bk_sb, kt_sb),
                ):
                    for t in range(TQ):  # 512-token chunks
                        xt = xin.tile([128, KS_D, 512], F16, tag="xt")
                        nc.sync.dma_start(
                            out=xt,
                            in_=xdram.rearrange("(k p) s -> p k s", p=128)[
                                :, :, t * 512 : (t + 1) * 512
                            ],
                        )
                        for m in range(2):  # output dim chunks of 128
                            ps = pj_ps.tile([128, 512], F32, tag="pj")
                            for ks in range(KS_D):
                                nc.tensor.matmul(
                                    ps,
                                    lhsT=w_sb[:, ks, m * 128 : (m + 1) * 128],
                                    rhs=xt[:, ks, :],
                                    start=(ks == 0),
                                    stop=(ks == KS_D - 1),
                                )
                            nc.scalar.activation(
                                out=qk_out(qk_sb, m, t),
                                in_=ps,
                                func=AF.Identity,
                                bias=b_sb[:, m : m + 1],
                            )

                # V projection: stationary x chunks, moving weights
                for t in range(TQ):
                    xt = xin.tile([128, KS_D, 512], F16, tag="xt")
                    nc.sync.dma_start(
                        out=xt,
                        in_=xv.rearrange("(k p) s -> p k s", p=128)[
                            :, :, t * 512 : (t + 1) * 512
                        ],
                    )
                    for i in range(4):  # 128-token subchunks
                        tc_i = t * 4 + i
                        ps = pj_ps.tile([128, DH], F32, tag="pjv")
                        for ks in range(KS_D):
                            nc.tensor.matmul(
                                ps,
                                lhsT=xt[:, ks, i * 128 : (i + 1) * 128],
                                rhs=wv_sb[:, ks, :],
                                start=(ks == 0),
                                stop=(ks == KS_D - 1),
                            )
                        for h in range(HPC):
                            nc.vector.tensor_add(
                                out=v_sb[tc_i][:, h, 0:DK],
                                in0=ps[:, h * DK : (h + 1) * DK],
                                in1=bvb[:, h * DK : (h + 1) * DK],
                            )
                        nc.vector.tensor_copy(
                            out=v_sb[tc_i][:, :, DK : DK + 1],
                            in_=one_col[:].broadcast_to([128, HPC, 1]),
                        )
                # QT pass last so attention can begin immediately after it
                for t in range(TQ):
                    xt = xin.tile([128, KS_D, 512], F16, tag="xt")
                    nc.sync.dma_start(
                        out=xt,
                        in_=xq.rearrange("(k p) s -> p k s", p=128)[
                            :, :, t * 512 : (t + 1) * 512
                        ],
                    )
                    for m in range(2):
                        ps = pj_ps.tile([128, 512], F32, tag="pj")
                        for ks in range(KS_D):
                            nc.tensor.matmul(
                                ps,
                                lhsT=wq_sb[:, ks, m * 128 : (m + 1) * 128],
                                rhs=xt[:, ks, :],
                                start=(ks == 0),
                                stop=(ks == KS_D - 1),
                            )
                        nc.scalar.activation(
                            out=qk_out(qt_sb, m, t),
                            in_=ps,
                            func=AF.Identity,
                            bias=bq_sb[:, m : m + 1],
                        )

            # ---- attention ----
            # Normalize/DMA of iteration i is emitted during iteration i+1's
            # compute (software pipeline) so PE never stalls on the DVE
            # reciprocal, and attn DMA spreads across the next kc loop.
            with tc.tile_pool(name="et", bufs=2 * KC + 2) as et_pool, \
                 tc.tile_pool(name="anorm", bufs=6) as anorm, \
                 tc.tile_pool(name="small", bufs=4) as small, \
                 tc.tile_pool(name="ctxst", bufs=2) as ctxst_pool, \
                 tc.tile_pool(name="st_ps", bufs=2, space="PSUM") as st_psp, \
                 tc.tile_pool(name="ctx_ps", bufs=1, space="PSUM") as ctx_psp, \
                 tc.tile_pool(name="bc_ps", bufs=1, space="PSUM") as bc_psp:

                def norm_head(state):
                    ets, recip, ctxst, h, qh = state
                    q0 = qh * QHW
                    pbase = (h % 2) * 64
                    mm = h // 2
                    bc_ps = bc_psp.tile([128, QHW], F32, tag="bc")
                    for j in range(QHW // 512):
                        nc.tensor.matmul(
                            bc_ps[:, j * 512 : (j + 1) * 512],
                            lhsT=ones_sb,
                            rhs=recip[:, j * 512 : (j + 1) * 512],
                            start=True,
                            stop=True,
                        )
                    bc_sb = small.tile([128, QHW], F16, tag="bc_sb")
                    nc.scalar.activation(out=bc_sb, in_=bc_ps, func=AF.Copy)
                    nc.vector.tensor_mul(
                        out=ctx_sb[pbase : pbase + 64, mm, q0 : q0 + QHW],
                        in0=ctxst,
                        in1=bc_sb[0:DK, :],
                    )
                    return bc_sb

                def norm_tail(state, bc_sb):
                    ets, recip, ctxst, h, qh = state
                    q0 = qh * QHW
                    for kc in range(KC):
                        at = anorm.tile([128, QHW], F16, tag="at")
                        nc.vector.tensor_mul(out=at, in0=ets[kc], in1=bc_sb)
                        nc.sync.dma_start(
                            out=attn_t[
                                h, kc * 128 : (kc + 1) * 128, q0 : q0 + QHW
                            ],
                            in_=at,
                        )

                def emit_norm(state):
                    norm_tail(state, norm_head(state))

                pending = None
                iters = [(h, qh) for h in range(HPC) for qh in range(QH)]
                for idx, (h, qh) in enumerate(iters):
                    pbase = (h % 2) * 64
                    mm = h // 2
                    q0 = qh * QHW
                    ctx_ps = ctx_psp.tile([DK + 1, QHW], F32, tag="ctx")
                    ets = []
                    for kc in range(KC):
                        st_ps = st_psp.tile([128, QHW], F32, tag="st")
                        for j in range(QHW // 512):
                            nc.tensor.matmul(
                                st_ps[:, j * 512 : (j + 1) * 512],
                                lhsT=kt_sb[mm][
                                    pbase : pbase + 64,
                                    kc * 128 : (kc + 1) * 128,
                                ],
                                rhs=qt_sb[mm][qh][
                                    pbase : pbase + 64,
                                    j * 512 : (j + 1) * 512,
                                ],
                                start=True,
                                stop=True,
                            )
                        et = et_pool.tile([128, QHW], F16, tag="et")
                        nc.scalar.activation(
                            out=et, in_=st_ps, func=AF.Exp, scale=SCALE
                        )
                        ets.append(et)
                        for j in range(QHW // 512):
                            nc.tensor.matmul(
                                ctx_ps[:, j * 512 : (j + 1) * 512],
                                lhsT=v_sb[kc][:, h, :],
                                rhs=et[:, j * 512 : (j + 1) * 512],
                                start=(kc == 0),
                                stop=(kc == KC - 1),
                            )

                    recip = small.tile([1, QHW], F32R, tag="recip")
                    nc.vector.reciprocal(out=recip, in_=ctx_ps[DK : DK + 1, :])
                    ctxst = ctxst_pool.tile([DK, QHW], F32, tag="cst")
                    nc.vector.tensor_copy(out=ctxst, in_=ctx_ps[0:DK, :])
                    state = (ets, recip, ctxst, h, qh)
                    if idx == len(iters) - 1:
                        # flush: publish last ctx slice first so the output
                        # projection unblocks, then drain both norm tails
                        bc_last = norm_head(state)
                        if pending is not None:
                            emit_norm(pending)
                        norm_tail(state, bc_last)
                        pending = None
                    else:
                        if pending is not None:
                            emit_norm(pending)
                        pending = state

            # ---- output projection (partial; host adds bo, sums cores) ----
            with tc.tile_pool(name="osb", bufs=6) as osb, \
                 tc.tile_pool(name="o_ps", bufs=4, space="PSUM") as o_psp:
                for t in range(TC):
                    for n in range(2):
                        ps = o_psp.tile([128, 512], F32, tag="o")
                        for ks in range(2):
                            nc.tensor.matmul(
                                ps,
                                lhsT=ctx_sb[:, ks, t * 128 : (t + 1) * 128],
                                rhs=wo_sb[:, ks, n * 512 : (n + 1) * 512],
                                start=(ks == 0),
                                stop=(ks == 1),
                            )
                        ot = osb.tile([128, 512], F16, tag="ot")
                        nc.scalar.activation(out=ot, in_=ps, func=AF.Copy)
                        nc.sync.dma_start(
                            out=out_p[
                                t * 128 : (t + 1) * 128, n * 512 : (n + 1) * 512
                            ],
                            in_=ot,
                        )

    return split_multi_waits(nc) if split_waits else nc


_NC_CACHE = None


def _get_nc():
    global _NC_CACHE
    if _NC_CACHE is None:
        _NC_CACHE = build_nc()
    return _NC_CACHE


def _shard_inputs(query, key, value, Wq, bq, Wk, bk, Wv, bv, Wo, bo):
    f32 = np.float32
    f16 = np.float16
    xqT = [np.ascontiguousarray(query[b].T, dtype=f16) for b in range(B)]
    xkT = [np.ascontiguousarray(key[b].T, dtype=f16) for b in range(B)]
    xvT = [np.ascontiguousarray(value[b].T, dtype=f16) for b in range(B)]
    in_maps = []
    for c in range(NCORES):
        b = c // 4
        r0 = (c % 4) * DH
        in_maps.append(
            {
                "xq": xqT[b],
                "xk": xkT[b],
                "xv": xvT[b],
                "wq": np.ascontiguousarray(Wq[r0 : r0 + DH, :].T, dtype=f16),
                "wk": np.ascontiguousarray(Wk[r0 : r0 + DH, :].T, dtype=f16),
                "wv": np.ascontiguousarray(Wv[r0 : r0 + DH, :].T, dtype=f16),
                "wo": np.ascontiguousarray(Wo[:, r0 : r0 + DH].T, dtype=f16),
                "bq": np.ascontiguousarray(bq[r0 : r0 + DH], dtype=f32),
                "bk": np.ascontiguousarray(bk[r0 : r0 + DH], dtype=f32),
                "bv": np.ascontiguousarray(bv[r0 : r0 + DH], dtype=f32),
            }
        )
    return in_maps


def kernel(query, key, value, Wq, bq, Wk, bk, Wv, bv, Wo, bo, _trace=False,
           _tmpdir=None):
    query, key, value = (np.asarray(a) for a in (query, key, value))
    nc = _get_nc()
    in_maps = _shard_inputs(query, key, value, Wq, bq, Wk, bk, Wv, bv, Wo, bo)
    res = run_bass_kernel_spmd(
        nc, in_maps, list(range(NCORES)), trace=_trace, tmpdir=_tmpdir
    )
    results = res.results

    attn = np.empty((B, NHEADS, S, S), dtype=np.float32)
    output = np.zeros((B, S, D), dtype=np.float32)
    for c in range(NCORES):
        b = c // 4
        h0 = (c % 4) * HPC
        at = results[c]["attn_t"]  # [HPC, S(k), S(q)]
        attn[b, h0 : h0 + HPC] = at.swapaxes(1, 2)
        output[b] += results[c]["out_p"]
    output += np.asarray(bo, dtype=np.float32)
    if _trace:
        return (output, attn), res
    return (output, attn)
